# revision 50
# baseline (speedup 1.0000x reference)
"""EdgeCNN (DGCNN) Bass/Tile kernel for TRN2 — one batch element per core.

Per edge-conv layer (N=1024 points, K=20 neighbors):
  1. PE: packed-key matmul  pd[n,j] = 2<xn,xj> - S[j] - S[n]   (PSUM, fp32;
     L4 runs its pd matmuls with fp16 inputs for 4x PE rate)
  2. DVE: one-pass (pd & ~0x3FF) | j  -> packed keys (scalar_tensor_tensor)
  3. DVE: 3x max8 + 2x match_replace -> top-20 packed keys; extract j
  4. tiles PAIR up through the idx chain: one PE transpose + one broadcast-
     read jTi8 replicate + one DRAM wrap write + one transpose read-back
     per pair (dma_gather wants idxs wrapped: partition = n%16, 8x repl)
  5. SWDGE dma_gather: 4 gathers per pair (tile x k-half), one per queue;
     descriptor gen (~8ns/idx) runs concurrently on 4 Q7 core-pairs.
     a = x @ (g~ Wn)^T rows; fp16 a-tables for L3/L4 halve gather bytes
  6. DVE reduce_max over k (fp16 TT max-tree for L3/L4)
  7. PE: transpose(m) + c-matmul (c = x @ (g~(Wc-Wn))^T + b) in PSUM
  8. ACT: leaky-relu (Prelu alpha=0.2) PSUM -> next layer xT
Head: conv5 (x1-x3 partials + bias precomputed under L4; x4 half in fp16),
global max-pool, 3 FC layers on PE.

NOTE: the schedule is very sensitive to the emission position of the big
weight-load DMAs (moving the FC loads earlier cost ~100us) and to engine
assignment of the idx-chain DMAs; measure 3+ reps before trusting a delta.
"""

import contextlib

import numpy as np

import concourse.bass as bass
import concourse.bacc as bacc
import concourse.mybir as mybir
from concourse.tile import TileContext
from concourse.masks import make_identity

F32 = mybir.dt.float32
U32 = mybir.dt.uint32
I16 = mybir.dt.int16
F16 = mybir.dt.float16
AF = mybir.ActivationFunctionType
ALU = mybir.AluOpType
AX = mybir.AxisListType

N = 1024
KNN = 20
NT = 8
NEG_SLOPE = 0.2
BNI = np.float32(1.0 / np.sqrt(1.0 + 1e-5))
LAYERS = [(3, 64), (64, 64), (64, 128), (128, 256)]
NEG_BIG = -3.0e38
NQ = 4  # SWDGE queues


def host_prep(inp):
    """Fold BN scale/bias into weights; transpose for device layout."""
    d = {}
    for li, (C, O) in enumerate(LAYERS, start=1):
        W = inp[f'W{li}'].astype(np.float32)
        g = inp[f'g{li}'].astype(np.float32)
        b = inp[f'b{li}'].astype(np.float32)
        gt = g * BNI
        Wn = W[:, :C]
        Wc = W[:, C:]
        d[f'wnt{li}'] = np.ascontiguousarray((gt[:, None] * Wn).T)          # (C, O)
        wdt = (gt[:, None] * (Wc - Wn)).T                                   # (C, O)
        if C + 2 <= 128:
            # bias folded in as an extra contraction row against the ones
            # row of the extended feature tensor
            d[f'wdt{li}'] = np.ascontiguousarray(
                np.vstack([wdt, b.reshape(1, O)]))                          # (C+1, O)
        else:
            d[f'wdt{li}'] = np.ascontiguousarray(wdt)                       # (C, O)
        d[f'bs{li}'] = b.reshape(1, O).copy()
    g5 = inp['g5'].astype(np.float32) * BNI
    d['w5t'] = np.ascontiguousarray((g5[:, None] * inp['W5']).T)            # (512, 512)
    # x4 half of conv5 runs in fp16 on device (x4 feeds only conv5)
    d['w5th'] = np.ascontiguousarray(d['w5t'][256:512].astype(np.float16))  # (256, 512)
    d['b5'] = inp['b5'].reshape(1, 512).astype(np.float32).copy()
    g1 = inp['bng1'].astype(np.float32) * BNI
    d['wfc1'] = np.ascontiguousarray((g1[:, None] * inp['fc1_w']).T)        # (512, 256)
    bf1 = g1 * inp['fc1_b'].astype(np.float32) + inp['bnb1'].astype(np.float32)
    d['bfc1'] = np.ascontiguousarray(bf1.reshape(2, 128).T)                 # (128, 2)
    g2 = inp['bng2'].astype(np.float32) * BNI
    d['wfc2'] = np.ascontiguousarray((g2[:, None] * inp['fc2_w']).T)        # (256, 128)
    bf2 = g2 * inp['fc2_b'].astype(np.float32) + inp['bnb2'].astype(np.float32)
    d['bfc2'] = np.ascontiguousarray(bf2.reshape(128, 1))                   # (128, 1)
    d['wfc3'] = np.ascontiguousarray(inp['fc3_w'].T)                        # (128, 40)
    d['bfc3'] = inp['fc3_b'].reshape(1, 40).astype(np.float32).copy()
    return d


def build_nc(stage='full'):
    # The stock cost model assumes the plain-SWDGE descriptor rate
    # (0.34 ns/desc); dma_gather's per-idx generation measures ~7.5 ns. The
    # Tile scheduler needs the real number to hide gathers behind compute.
    import concourse.hw_specs as hw_specs
    hw_specs.TRN2Spec.SWDGE_NS_PER_DESCRIPTOR = 7.5
    nc = bacc.Bacc("TRN2", target_bir_lowering=False, debug=False, num_devices=8,
                   num_swdge_queues=NQ)
    with TileContext(nc) as tc:
        _trace(nc, tc, stage)
    nc.compile()
    return nc


def _trace(nc, tc, stage='full'):
    with contextlib.ExitStack() as ctx:
        dram = ctx.enter_context(tc.tile_pool(name="dram", bufs=1, space="DRAM"))
        consts = ctx.enter_context(tc.tile_pool(name="consts", bufs=1))
        persist = ctx.enter_context(tc.tile_pool(name="persist", bufs=1))
        sb = ctx.enter_context(tc.tile_pool(name="sb", bufs=2))
        keyp = ctx.enter_context(tc.tile_pool(name="keyp", bufs=2))
        gath = ctx.enter_context(tc.tile_pool(name="gath", bufs=3))
        mp = ctx.enter_context(tc.tile_pool(name="mp", bufs=1))
        scr = ctx.enter_context(tc.tile_pool(name="scr", bufs=2))
        psb = ctx.enter_context(tc.tile_pool(name="psb", bufs=3, space="PSUM"))
        pss = ctx.enter_context(tc.tile_pool(name="pss", bufs=2, space="PSUM"))

        # ---- DRAM I/O ----
        x_d = dram.tile([3, N], F32, kind="ExternalInput", uniquify=False, name="x")
        win = {}
        for li, (C, O) in enumerate(LAYERS, start=1):
            wdtr = C + 1 if C + 2 <= 128 else C
            win[f'wnt{li}'] = dram.tile([C, O], F32, kind="ExternalInput", uniquify=False, name=f"wnt{li}")
            win[f'wdt{li}'] = dram.tile([wdtr, O], F32, kind="ExternalInput", uniquify=False, name=f"wdt{li}")
            win[f'bs{li}'] = dram.tile([1, O], F32, kind="ExternalInput", uniquify=False, name=f"bs{li}")
        w5t_d = dram.tile([512, 512], F32, kind="ExternalInput", uniquify=False, name="w5t")
        w5th_d = dram.tile([256, 512], F16, kind="ExternalInput", uniquify=False, name="w5th")
        b5_d = dram.tile([1, 512], F32, kind="ExternalInput", uniquify=False, name="b5")
        wfc1_d = dram.tile([512, 256], F32, kind="ExternalInput", uniquify=False, name="wfc1")
        bfc1_d = dram.tile([128, 2], F32, kind="ExternalInput", uniquify=False, name="bfc1")
        wfc2_d = dram.tile([256, 128], F32, kind="ExternalInput", uniquify=False, name="wfc2")
        bfc2_d = dram.tile([128, 1], F32, kind="ExternalInput", uniquify=False, name="bfc2")
        wfc3_d = dram.tile([128, 40], F32, kind="ExternalInput", uniquify=False, name="wfc3")
        bfc3_d = dram.tile([1, 40], F32, kind="ExternalInput", uniquify=False, name="bfc3")
        out_d = dram.tile([40, 1], F32, kind="ExternalOutput", uniquify=False, name="out")
        dbg_d = None
        if stage != 'full':
            dbg_d = dram.tile([128, N], F32, kind="ExternalOutput", uniquify=False, name="dbg")

        a_ds = {li: dram.tile([N, O], F16 if li >= 3 else F32, name=f"a_d{li}")
                for li, (C, O) in enumerate(LAYERS, start=1)}
        jw_ds = {li: dram.tile([N * KNN // 16, 128], I16, name=f"jw_d{li}")
                 for li in range(1, 5)}

        # ---- consts ----
        iotaJ = consts.tile([128, N], U32, tag="iotaJ")
        nc.gpsimd.iota(iotaJ[:, :], [[1, N]], base=0, channel_multiplier=0)
        ident = consts.tile([128, 128], F32, tag="ident")
        make_identity(nc, ident[:, :])
        onescol = consts.tile([128, 1], F32, tag="onescol")
        nc.vector.memset(onescol[:, :], 1.0)
        onesrow = consts.tile([1, N], F32, tag="onesrow")
        nc.vector.memset(onesrow[:, :], 1.0)
        maskc = consts.tile([128, 1], U32, tag="maskc")
        nc.vector.memset(maskc[:, :], 0xFFFFFC00)
        onesrow16 = consts.tile([1, N], F16, tag="onesrow16")
        nc.vector.memset(onesrow16[:, :], 1.0)

        # persistent feature tensors; layers with C+2 <= 128 carry two extra
        # rows [ones; -S] so the pd rank-2 terms fuse into the main matmul
        x0T = persist.tile([5, N], F32, tag="x0T")
        x1T = persist.tile([66, N], F32, tag="x1T")
        x2T = persist.tile([66, N], F32, tag="x2T")
        x3T = persist.tile([128, N], F32, tag="x3T")
        x4Ta = persist.tile([128, N], F16, tag="x4Ta")
        x4Tb = persist.tile([128, N], F16, tag="x4Tb")
        nc.sync.dma_start(x0T[3:4, :], onesrow[0:1, :])
        nc.sync.dma_start(x1T[64:65, :], onesrow[0:1, :])
        nc.sync.dma_start(x2T[64:65, :], onesrow[0:1, :])

        # x is pre-transposed on host to (3, N)
        nc.sync.dma_start(x0T[0:3, :], x_d[:, :])
        if stage == 'xload':
            nc.sync.dma_start(dbg_d[0:3, :], x0T[:, :])
            nc.sync.dma_start(out_d[:, :], onescol[0:40, :])
            return

        # weight loads are emitted per-layer via load_w: L1 up-front, the
        # rest inside the previous layer's gather phase so the sync engine's
        # DMA queue never delays a layer-head critical transfer
        wsb = {}

        def load_w(li):
            C, O = LAYERS[li - 1]
            fuse = C + 2 <= 128
            wnt = consts.tile([C, O], F32, tag=f"wnt{li}s")
            wdt = consts.tile([C + 1 if fuse else C, O], F32, tag=f"wdt{li}s")
            bs = consts.tile([1, O], F32, tag=f"bs{li}s")
            nc.sync.dma_start(wnt[:, :], win[f'wnt{li}'][:, :])
            nc.sync.dma_start(wdt[:, :], win[f'wdt{li}'][:, :])
            nc.sync.dma_start(bs[:, :], win[f'bs{li}'][:, :])
            wsb[li] = (wnt, wdt, bs)

        load_w(1)

        def edge_layer(li, xT, C, O, out_parts, post_tiles=None,
                       post_head=None):
            fuse = C + 2 <= 128
            a_d = a_ds[li]
            jw_d = jw_ds[li]
            jwap = jw_d[:, :]
            wnt, wdt, bs = wsb[li]

            xsq = sb.tile([C, N], F32, tag="xsq")
            for hh in range(2):
                hcols = slice(hh * 512, (hh + 1) * 512)
                nc.scalar.activation(xsq[:, hcols], xT[0:C, hcols], AF.Square)
            # x2e = [2x; -S; 1] (fused, fp32) or [2x] (L4, fp16: the L4 pd
            # matmuls run in fp16 for 4x PE rate; keys still pack from the
            # fp32 PSUM accumulation)
            x2e = sb.tile([C + 2 if fuse else C, N],
                          F32 if fuse else F16, tag="x2d")
            for hh in range(2):
                hcols = slice(hh * 512, (hh + 1) * 512)
                nc.scalar.activation(
                    x2e[0:C, hcols], xT[0:C, hcols], AF.Copy, bias=0.0, scale=2.0)
            negS = mp.tile([1, N], F32, tag="negS")
            if fuse:
                nc.sync.dma_start(x2e[C + 1:C + 2, :], onesrow[0:1, :])
            else:
                SO2 = mp.tile([2, N], F16, tag="SO2")
                OS2 = mp.tile([2, N], F16, tag="OS2")
                nc.vector.memset(OS2[0:1, :], -1.0)
                x3h = sb.tile([128, N], F16, tag="x3h")
                for hh in range(2):
                    hcols = slice(hh * 512, (hh + 1) * 512)
                    nc.scalar.activation(
                        x3h[:, hcols], xT[0:C, hcols], AF.Copy)
                negS16 = mp.tile([1, N], F16, tag="negS16")
            for h in range(2):
                cols = slice(h * 512, (h + 1) * 512)
                S_ps = pss.tile([1, 512], F32, tag="a")
                nc.tensor.matmul(
                    S_ps[:, :], lhsT=onescol[0:C, :], rhs=xsq[:, cols],
                    start=True, stop=True, skip_group_check=True)
                nc.scalar.activation(
                    negS[0:1, cols], S_ps[:, :], AF.Copy, bias=0.0, scale=-1.0)
                if not fuse:
                    nc.scalar.activation(
                        SO2[0:1, cols], S_ps[:, :], AF.Copy, bias=0.0, scale=1.0)
            if fuse:
                # compute engines can only address partition bases 0/32/64/96;
                # rows C/C+1 land elsewhere, so move -S in via DMA
                nc.sync.dma_start(xT[C + 1:C + 2, :], negS[0:1, :])
                if C % 32 == 0:
                    nc.scalar.activation(
                        x2e[C:C + 1, :], negS[0:1, :], AF.Copy)
                else:
                    nc.sync.dma_start(x2e[C:C + 1, :], negS[0:1, :])
            else:
                nc.scalar.activation(negS16[0:1, :], negS[0:1, :], AF.Copy)
                nc.sync.dma_start(SO2[1:2, :], onesrow16[0:1, :])
                nc.sync.dma_start(OS2[1:2, :], negS16[0:1, :])

            # a-rows to DRAM first so gathers can start as soon as idx ready
            for t in range(NT):
                a_ps = pss.tile([128, O], F32, tag="a")
                nc.tensor.matmul(
                    a_ps[:, :], lhsT=xT[0:C, t * 128:(t + 1) * 128], rhs=wnt[:, :],
                    start=True, stop=True, skip_group_check=True)
                a_sb = sb.tile([128, O], F16 if li >= 3 else F32, tag="a_sb")
                nc.scalar.activation(a_sb[:, :], a_ps[:, :], AF.Copy)
                nc.sync.dma_start(a_d[t * 128:(t + 1) * 128, :], a_sb[:, :])

            if post_head is not None:
                post_head()

            m = mp.tile([128, NT, O], F32, tag="m")
            pending_g = []

            # c = x @ (g~(Wc-Wn))^T + b depends only on xT, so it runs on PE
            # during the gather phase and lands in SBUF for the tail TT-add
            csb = {}

            def emit_c():
                for ot in range(len(out_parts)):
                    orow = out_parts[ot][1]
                    cs = sb.tile([128, N], F32, tag=f"cs{ot}", bufs=1)
                    for h in range(2):
                        cols = slice(h * 512, (h + 1) * 512)
                        c_ps = pss.tile([128, 512], F32, tag="a")
                        if fuse:
                            # wdt carries the bias as row C vs xT's ones row
                            nc.tensor.matmul(
                                c_ps[0:orow, :],
                                lhsT=wdt[:, ot * 128:ot * 128 + orow],
                                rhs=xT[0:C + 1, cols],
                                start=True, stop=True, skip_group_check=True)
                        else:
                            nc.tensor.matmul(
                                c_ps[0:orow, :],
                                lhsT=wdt[:, ot * 128:ot * 128 + orow],
                                rhs=xT[:, cols],
                                start=True, stop=False, skip_group_check=True)
                            nc.tensor.matmul(
                                c_ps[0:orow, :],
                                lhsT=bs[0:1, ot * 128:ot * 128 + orow],
                                rhs=onesrow[0:1, cols],
                                start=False, stop=True, skip_group_check=True)
                        nc.scalar.activation(
                            cs[0:orow, cols], c_ps[0:orow, :], AF.Copy)
                    csb[ot] = cs

            def emit_reduce(tp, gp, koff):
                if li >= 3:
                    # fp16: contiguous TT max tree (2x DVE mode), in place
                    t10 = sb.tile([128, 10, O], F16, tag="t10", bufs=2)
                    nc.vector.tensor_tensor(
                        out=t10[:, :, :], in0=gp[:, koff:koff + 10, :],
                        in1=gp[:, koff + 10:koff + 20, :], op=ALU.max)
                    nc.vector.tensor_tensor(
                        out=t10[:, 0:5, :], in0=t10[:, 0:5, :], in1=t10[:, 5:10, :],
                        op=ALU.max)
                    nc.vector.tensor_tensor(
                        out=t10[:, 0:2, :], in0=t10[:, 0:2, :], in1=t10[:, 2:4, :],
                        op=ALU.max)
                    nc.vector.tensor_tensor(
                        out=t10[:, 0, :], in0=t10[:, 0, :], in1=t10[:, 1, :],
                        op=ALU.max)
                    nc.vector.tensor_tensor(
                        out=m[:, tp, :], in0=t10[:, 0, :], in1=t10[:, 4, :],
                        op=ALU.max)
                    return
                gap = gp[:, :, :]
                red0 = bass.AP(gap.tensor, gap.offset + koff * O,
                               [gap.ap[0], [1, O], [O, KNN // 2]])
                red1 = bass.AP(gap.tensor, gap.offset + (koff + KNN // 2) * O,
                               [gap.ap[0], [1, O], [O, KNN // 2]])
                mtmp = sb.tile([128, O], F32, tag="mtmp", bufs=2)
                nc.vector.tensor_reduce(
                    out=m[:, tp, :], in_=red0, axis=AX.X, op=ALU.max)
                nc.vector.tensor_reduce(
                    out=mtmp[:, :], in_=red1, axis=AX.X, op=ALU.max)
                nc.vector.tensor_tensor(
                    out=m[:, tp, :], in0=m[:, tp, :], in1=mtmp[:, :],
                    op=ALU.max)

            def emit_pair(tp0, tp1, gp):
                emit_reduce(tp0, gp, 0)
                emit_reduce(tp1, gp, KNN)
            ntl = NT
            if stage.startswith('topk1_'):
                ntl = int(stage.split('_')[1])
            for t in range(ntl):
                if stage in ('keys', 'pack', 'max1', 'mr1', 'ext', 'topkt0') and t > 0:
                    break
                tcols = slice(t * 128, (t + 1) * 128)
                kp = psb.tile([128, N], F32, tag="big")
                for h in range(2):
                    cols = slice(h * 512, (h + 1) * 512)
                    if fuse:
                        nc.tensor.matmul(
                            kp[:, cols], lhsT=xT[:, tcols], rhs=x2e[:, cols],
                            start=True, stop=True, skip_group_check=True)
                    else:
                        nc.tensor.matmul(
                            kp[:, cols], lhsT=x3h[:, tcols], rhs=x2e[:, cols],
                            start=True, stop=False, skip_group_check=True)
                        nc.tensor.matmul(
                            kp[:, cols], lhsT=SO2[:, tcols], rhs=OS2[:, cols],
                            start=False, stop=True, skip_group_check=True)
                if stage == 'keys' and t == 0:
                    kcp = sb.tile([128, N], F32, tag="kcp")
                    nc.scalar.activation(kcp[:, :], kp[:, :], AF.Copy)
                    nc.sync.dma_start(dbg_d[:, :], kcp[:, :])
                    nc.sync.dma_start(out_d[:, :], onescol[0:40, :])
                    return 'stop'
                kb = keyp.tile([128, N], U32, tag="keysP")
                nc.vector.scalar_tensor_tensor(
                    out=kb[:, :], in0=kp[:, :].bitcast(U32), scalar=maskc[:, 0:1],
                    in1=iotaJ[:, :], op0=ALU.bitwise_and, op1=ALU.bitwise_or)
                if stage == 'pack' and t == 0:
                    kcp = sb.tile([128, N], F32, tag="kcp")
                    nc.vector.tensor_copy(kcp[:, :].bitcast(U32), kb[:, :])
                    nc.sync.dma_start(dbg_d[:, :], kcp[:, :])
                    nc.sync.dma_start(out_d[:, :], onescol[0:40, :])
                    return 'stop'
                kbf = kb[:, :].bitcast(F32)
                v64 = scr.tile([128, 64], F32, tag="v64")
                for ch in range(8):
                    nc.vector.max(
                        v64[:, ch * 8:(ch + 1) * 8],
                        kb[:, ch * 128:(ch + 1) * 128].bitcast(F32))
                v24 = scr.tile([128, 24], F32, tag="v24")
                nc.vector.max(v24[:, 0:8], v64[:, :])
                if stage == 'max1' and t == 0:
                    nc.sync.dma_start(dbg_d[:, 0:8], v24[:, 0:8])
                    nc.sync.dma_start(out_d[:, :], onescol[0:40, :])
                    return 'stop'
                v64f = v64[:, :]
                nc.vector.match_replace(v64f, v24[:, 0:8], v64f, NEG_BIG)
                if stage == 'mr1' and t == 0:
                    nc.sync.dma_start(dbg_d[:, :], kb[:, :].bitcast(F32))
                    nc.sync.dma_start(out_d[:, :], onescol[0:40, :])
                    return 'stop'
                nc.vector.max(v24[:, 8:16], v64f)
                nc.vector.match_replace(v64f, v24[:, 8:16], v64f, NEG_BIG)
                nc.vector.max(v24[:, 16:24], v64f)
                if stage == 'ext' and t == 0:
                    nc.sync.dma_start(dbg_d[:, 0:24], v24[:, :])
                    nc.sync.dma_start(out_d[:, :], onescol[0:40, :])
                    return 'stop'
                j20 = scr.tile([128, KNN], U32, tag="j20")
                nc.vector.tensor_scalar(
                    j20[:, :], v24[:, 0:KNN].bitcast(U32), 0x3FF, None,
                    op0=ALU.bitwise_and)
                # j as fp32 values; two tiles share one jf2 so the whole idx
                # chain (transpose/replicate/DRAM wrap/gather) runs once per
                # tile PAIR, halving per-tile sequencer round-trips
                if t % 2 == 0:
                    jf2 = scr.tile([128, 2, KNN], F32, tag="jf", bufs=2)
                nc.vector.tensor_copy(jf2[:, t % 2, :], j20[:, :])
                if stage == 'topkt0' and t == 0:
                    nc.sync.dma_start(dbg_d[:, 0:KNN], jf2[:, 0, :])
                    nc.sync.dma_start(out_d[:, :], onescol[0:40, :])
                    return 'stop'
                if t % 2 == 0:
                    continue

                # pair idx wrap + 4 full-size gathers (one per queue)
                jT_ps = pss.tile([2 * KNN, 128], F32, tag="a")
                nc.tensor.matmul(
                    jT_ps[:, :], lhsT=jf2[:, :, :].rearrange("p a k -> p (a k)"),
                    rhs=ident[:, 0:128], is_transpose=True, start=True, stop=True,
                    skip_group_check=True)
                # build the 8x-replicated row layout in one activation via a
                # stride-0 (broadcast) read dim, then a single DMA write
                jTi8 = sb.tile([2 * KNN, 1024], I16, tag="jTi8", bufs=2)
                pap = jT_ps[:, :]
                rep_src = bass.AP(pap.tensor, pap.offset,
                                  [pap.ap[0], [16, 8], [0, 8], [1, 16]])
                nc.scalar.activation(jTi8[:, :], rep_src, AF.Copy)
                dst = bass.AP(jwap.tensor, jwap.offset + (t - 1) * 160 * 128,
                              [[1024, 2 * KNN], [1, 1024]])
                nc.sync.dma_start(dst, jTi8[:, :])
                src_ap = bass.AP(jwap.tensor, jwap.offset + (t - 1) * 160 * 128,
                                 [[128, 320], [1, 128]])
                idq = keyp.tile([128, 320], I16, tag="idxq", bufs=2)
                nc.scalar.dma_start_transpose(idq[:, :], src_ap)

                gdt = F16 if li >= 3 else F32
                g = gath.tile([128, 2 * KNN, O], gdt, tag="g", bufs=3)
                # 4 gathers per pair: (tile, k-half) x (2, 2), one per queue;
                # each idx range is contiguous in idq
                for gh in range(4):
                    tt, kh = gh // 2, gh % 2
                    nc.gpsimd.dma_gather(
                        out_ap=g[:, tt * KNN + kh * 10:tt * KNN + kh * 10 + 10, :],
                        in_ap=a_d[:, :],
                        idxs_ap=idq[:, tt * 160 + kh * 80:tt * 160 + kh * 80 + 80],
                        num_idxs=1280, num_idxs_reg=1280, elem_size=O,
                        single_packet=False,
                        queue_num=(2 * t + gh) % NQ)
                pending_g.append((t - 1, t, g))
                # emit gather-independent PE work (c-matmuls, zpart_fill)
                # mid-loop so the scheduler anchors it INSIDE the gather
                # phase, not after it
                if t == 1:
                    emit_c()
                if post_tiles is not None and t == 3:
                    post_tiles()
                    post_tiles = None
                # lag the reduce a couple pairs so the DVE stream never
                # head-of-line blocks on an in-flight gather
                if len(pending_g) > 2:
                    emit_pair(*pending_g.pop(0))

            for tp0, tp1, gp in pending_g:
                emit_pair(tp0, tp1, gp)

            if stage.startswith('topk1'):
                nc.sync.dma_start(dbg_d[0:128, 0:KNN], jf2[:, 1, :])
                nc.sync.dma_start(out_d[:, :], onescol[0:40, :])
                return 'stop'

            if post_tiles is not None:
                post_tiles()

            # transpose m + add precomputed c + lrelu -> out_parts; the c
            # part was computed into SBUF during the gather phase (emit_c),
            # so the tail carries only transposes + TT-add + Prelu
            for ot, (otile, orow) in enumerate(out_parts):
                px = psb.tile([orow, N], F32, tag="big")
                for h in range(2):
                    cols = slice(h * 512, (h + 1) * 512)
                    for t in range(4 * h, 4 * h + 4):
                        nc.tensor.matmul(
                            px[:, t * 128:(t + 1) * 128],
                            lhsT=m[:, t, ot * 128:ot * 128 + orow],
                            rhs=ident[:, 0:128],
                            is_transpose=True, start=(t % 4 == 0),
                            stop=(t % 4 == 3), skip_group_check=True)
                    nc.vector.tensor_tensor(
                        out=px[:, cols], in0=px[:, cols],
                        in1=csb[ot][0:orow, cols], op=ALU.add)
                    nc.scalar.activation(
                        otile[0:orow, cols], px[:, cols], AF.Prelu,
                        alpha=NEG_SLOPE)

        w5sb = {}
        b5sb_box = {}

        def load_w5():
            for ci, (rows, k0) in enumerate([(64, 0), (64, 64), (128, 128)]):
                w5c = consts.tile([rows, 512], F32, tag=f"w5c{ci}")
                nc.sync.dma_start(w5c[:, :], w5t_d[k0:k0 + rows, :])
                w5sb[ci] = w5c
            for ci, k0 in ((3, 0), (4, 128)):
                w5c = consts.tile([128, 512], F16, tag=f"w5c{ci}")
                nc.sync.dma_start(w5c[:, :], w5th_d[k0:k0 + 128, :])
                w5sb[ci] = w5c
            b5sb = consts.tile([1, 512], F32, tag="b5sb")
            nc.sync.dma_start(b5sb[:, :], b5_d[:, :])
            b5sb_box[0] = b5sb

        fcsb = {}

        def load_fc():
            wfc1sb = consts.tile([128, 4, 256], F32, tag="wfc1sb")
            for c in range(4):
                nc.sync.dma_start(wfc1sb[:, c, :],
                                  wfc1_d[c * 128:(c + 1) * 128, :])
            bfc1sb = consts.tile([128, 2], F32, tag="bfc1sb")
            nc.sync.dma_start(bfc1sb[:, :], bfc1_d[:, :])
            wfc2sb = consts.tile([128, 2, 128], F32, tag="wfc2sb")
            for c in range(2):
                nc.sync.dma_start(wfc2sb[:, c, :],
                                  wfc2_d[c * 128:(c + 1) * 128, :])
            bfc2sb = consts.tile([128, 1], F32, tag="bfc2sb")
            nc.sync.dma_start(bfc2sb[:, :], bfc2_d[:, :])
            wfc3sb = consts.tile([128, 40], F32, tag="wfc3sb")
            nc.sync.dma_start(wfc3sb[:, :], wfc3_d[:, :])
            bfc3sb = consts.tile([1, 40], F32, tag="bfc3sb")
            nc.sync.dma_start(bfc3sb[:, :], bfc3_d[:, :])
            fcsb.update(wfc1sb=wfc1sb, bfc1sb=bfc1sb, wfc2sb=wfc2sb,
                        bfc2sb=bfc2sb, wfc3sb=wfc3sb, bfc3sb=bfc3sb)

        zpart = persist.tile([128, NT, 512], F32, tag="zpart")

        def zpart_fill():
            for t in range(NT):
                tcols = slice(t * 128, (t + 1) * 128)
                zp_ps = pss.tile([128, 512], F32, tag="a")
                for ci, (xt, rows) in enumerate(
                        [(x1T, 64), (x2T, 64), (x3T, 128)]):
                    nc.tensor.matmul(
                        zp_ps[:, :], lhsT=xt[0:rows, tcols], rhs=w5sb[ci][:, :],
                        start=(ci == 0), stop=False, skip_group_check=True)
                # conv5 bias folded in here (hidden under L4) instead of in
                # the head's critical path
                nc.tensor.matmul(
                    zp_ps[:, :], lhsT=onesrow[0:1, tcols],
                    rhs=b5sb_box[0][:, :], start=False, stop=True,
                    skip_group_check=True)
                nc.scalar.activation(zpart[:, t, :], zp_ps[:, :], AF.Copy)

        load_w(2)
        load_w(3)
        load_w(4)
        load_w5()
        r = edge_layer(1, x0T, 3, 64, [(x1T, 64)])
        if r == 'stop':
            return
        if stage == 'gath1':
            nc.sync.dma_start(dbg_d[0:64, :], x1T[0:64, :])
            nc.sync.dma_start(out_d[:, :], onescol[0:40, :])
            return
        edge_layer(2, x1T, 64, 64, [(x2T, 64)])
        edge_layer(3, x2T, 64, 128, [(x3T, 128)])
        edge_layer(4, x3T, 128, 256, [(x4Ta, 128), (x4Tb, 128)],
                   post_tiles=zpart_fill)

        # ---- head: conv5 (x4 chunks; x1-x3 partials precomputed) + max pool ----
        zmax = persist.tile([128, 512], F32, tag="zmax")
        for t in range(NT):
            tcols = slice(t * 128, (t + 1) * 128)
            z_ps = pss.tile([128, 512], F32, tag="a")
            for ci, (xt, rows, k0) in enumerate(
                    [(x4Ta, 128, 256), (x4Tb, 128, 384)]):
                nc.tensor.matmul(
                    z_ps[:, :], lhsT=xt[:, tcols], rhs=w5sb[3 + ci][:, :],
                    start=(ci == 0), stop=(ci == 1), skip_group_check=True)
            zsb = sb.tile([128, 512], F32, tag="zsb")
            nc.vector.tensor_tensor(
                out=zsb[:, :], in0=zpart[:, t, :], in1=z_ps[:, :], op=ALU.add)
            if t == 0:
                nc.scalar.activation(zmax[:, :], zsb[:, :], AF.Copy)
            else:
                nc.vector.tensor_tensor(
                    out=zmax[:, :], in0=zmax[:, :], in1=zsb[:, :], op=ALU.max)
        # partition tree-max 128 -> 1... then we need yT [128, 4] instead:
        # transpose zmax chunks and reduce along free dim.
        yT = persist.tile([128, 4], F32, tag="yT")
        for cchunk in range(4):
            zt_ps = pss.tile([128, 128], F32, tag="a")
            nc.tensor.matmul(
                zt_ps[:, :], lhsT=zmax[:, cchunk * 128:(cchunk + 1) * 128],
                rhs=ident[:, 0:128], is_transpose=True, start=True, stop=True,
                skip_group_check=True)
            nc.vector.tensor_reduce(
                out=yT[:, cchunk:cchunk + 1], in_=zt_ps[:, :],
                axis=AX.X, op=ALU.max)
        # leaky relu on yT
        yTr = persist.tile([128, 4], F32, tag="yTr")
        nc.scalar.activation(yTr[:, :], yT[:, :], AF.Prelu, alpha=NEG_SLOPE)

        # ---- FC head ----
        load_fc()
        wfc1sb = fcsb['wfc1sb']
        bfc1sb = fcsb['bfc1sb']
        wfc2sb = fcsb['wfc2sb']
        bfc2sb = fcsb['bfc2sb']
        wfc3sb = fcsb['wfc3sb']
        bfc3sb = fcsb['bfc3sb']

        h1sb = persist.tile([128, 2], F32, tag="h1sb")
        for mt in range(2):
            h1_ps = pss.tile([128, 1], F32, tag="a")
            for c in range(4):
                nc.tensor.matmul(
                    h1_ps[:, :], lhsT=wfc1sb[:, c, mt * 128:(mt + 1) * 128],
                    rhs=yTr[:, c:c + 1],
                    start=(c == 0), stop=(c == 3), skip_group_check=True)
            nc.scalar.activation(
                h1sb[:, mt:mt + 1], h1_ps[:, :], AF.Prelu,
                bias=bfc1sb[:, mt:mt + 1], scale=1.0, alpha=NEG_SLOPE)
        h2sb = persist.tile([128, 1], F32, tag="h2sb")
        h2_ps = pss.tile([128, 1], F32, tag="a")
        for c in range(2):
            nc.tensor.matmul(
                h2_ps[:, :], lhsT=wfc2sb[:, c, :], rhs=h1sb[:, c:c + 1],
                start=(c == 0), stop=(c == 1), skip_group_check=True)
        nc.scalar.activation(
            h2sb[:, :], h2_ps[:, :], AF.Prelu,
            bias=bfc2sb[:, :], scale=1.0, alpha=NEG_SLOPE)

        out_ps = pss.tile([40, 1], F32, tag="a")
        nc.tensor.matmul(
            out_ps[:, :], lhsT=wfc3sb[:, :], rhs=h2sb[:, :],
            start=True, stop=False, skip_group_check=True)
        nc.tensor.matmul(
            out_ps[:, :], lhsT=bfc3sb[:, :], rhs=onescol[0:1, :],
            start=False, stop=True, skip_group_check=True)
        out_sb = persist.tile([40, 1], F32, tag="out_sb")
        nc.scalar.activation(out_sb[:, :], out_ps[:, :], AF.Copy)
        nc.sync.dma_start(out_d[:, :], out_sb[:, :])


# ---------------------------------------------------------------------------
# harness entry point
# ---------------------------------------------------------------------------
_NC_CACHE = {}


def _get_nc():
    if 'nc' not in _NC_CACHE:
        _NC_CACHE['nc'] = build_nc()
    return _NC_CACHE['nc']


def kernel(**inputs):
    """Full-batch EdgeCNN forward. x: (8, 1024, 3) -> (8, 40) float32.

    Pure data parallel: batch element b runs on NeuronCore b.
    """
    from concourse.bass_utils import run_bass_kernel_spmd

    inp = {k: np.asarray(v) for k, v in inputs.items()}
    prep = host_prep(inp)
    nc = _get_nc()
    in_maps = []
    for b in range(8):
        m = {'x': np.ascontiguousarray(inp['x'][b].T).astype(np.float32)}
        m.update(prep)
        in_maps.append(m)
    res = run_bass_kernel_spmd(nc, in_maps, core_ids=list(range(8)))
    out = np.stack([res.results[b]['out'].reshape(40) for b in range(8)])
    return out.astype(np.float32)



# revision 51
# speedup vs baseline: 1.0014x; 1.0014x over previous
"""EdgeCNN (DGCNN) Bass/Tile kernel for TRN2 — one batch element per core.

Per edge-conv layer (N=1024 points, K=20 neighbors):
  1. PE: packed-key matmul  pd[n,j] = 2<xn,xj> - S[j] - S[n]   (PSUM, fp32;
     L4 runs its pd matmuls with fp16 inputs for 4x PE rate)
  2. DVE: one-pass (pd & ~0x3FF) | j  -> packed keys (scalar_tensor_tensor)
  3. DVE: 3x max8 + 2x match_replace -> top-20 packed keys; extract j
  4. tiles PAIR up through the idx chain: one PE transpose + one broadcast-
     read jTi8 replicate + one DRAM wrap write + one transpose read-back
     per pair (dma_gather wants idxs wrapped: partition = n%16, 8x repl)
  5. SWDGE dma_gather: 4 gathers per pair (tile x k-half), one per queue;
     descriptor gen (~8ns/idx) runs concurrently on 4 Q7 core-pairs.
     a = x @ (g~ Wn)^T rows; fp16 a-tables for L3/L4 halve gather bytes
  6. DVE reduce_max over k (fp16 TT max-tree for L3/L4)
  7. PE: transpose(m) + c-matmul (c = x @ (g~(Wc-Wn))^T + b) in PSUM
  8. ACT: leaky-relu (Prelu alpha=0.2) PSUM -> next layer xT
Head: conv5 (x1-x3 partials + bias precomputed under L4; x4 half in fp16),
global max-pool, 3 FC layers on PE.

NOTE: the schedule is very sensitive to the emission position of the big
weight-load DMAs (moving the FC loads earlier cost ~100us) and to engine
assignment of the idx-chain DMAs; measure 3+ reps before trusting a delta.
"""

import contextlib

import numpy as np

import concourse.bass as bass
import concourse.bacc as bacc
import concourse.mybir as mybir
from concourse.tile import TileContext
from concourse.masks import make_identity

F32 = mybir.dt.float32
U32 = mybir.dt.uint32
I16 = mybir.dt.int16
F16 = mybir.dt.float16
AF = mybir.ActivationFunctionType
ALU = mybir.AluOpType
AX = mybir.AxisListType

N = 1024
KNN = 20
NT = 8
NEG_SLOPE = 0.2
BNI = np.float32(1.0 / np.sqrt(1.0 + 1e-5))
LAYERS = [(3, 64), (64, 64), (64, 128), (128, 256)]
NEG_BIG = -3.0e38
NQ = 4  # SWDGE queues


def host_prep(inp):
    """Fold BN scale/bias into weights; transpose for device layout."""
    d = {}
    for li, (C, O) in enumerate(LAYERS, start=1):
        W = inp[f'W{li}'].astype(np.float32)
        g = inp[f'g{li}'].astype(np.float32)
        b = inp[f'b{li}'].astype(np.float32)
        gt = g * BNI
        Wn = W[:, :C]
        Wc = W[:, C:]
        d[f'wnt{li}'] = np.ascontiguousarray((gt[:, None] * Wn).T)          # (C, O)
        wdt = (gt[:, None] * (Wc - Wn)).T                                   # (C, O)
        if C + 2 <= 128:
            # bias folded in as an extra contraction row against the ones
            # row of the extended feature tensor
            d[f'wdt{li}'] = np.ascontiguousarray(
                np.vstack([wdt, b.reshape(1, O)]))                          # (C+1, O)
        else:
            d[f'wdt{li}'] = np.ascontiguousarray(wdt)                       # (C, O)
        d[f'bs{li}'] = b.reshape(1, O).copy()
    g5 = inp['g5'].astype(np.float32) * BNI
    d['w5t'] = np.ascontiguousarray((g5[:, None] * inp['W5']).T)            # (512, 512)
    # x4 half of conv5 runs in fp16 on device (x4 feeds only conv5)
    d['w5th'] = np.ascontiguousarray(d['w5t'][256:512].astype(np.float16))  # (256, 512)
    d['b5'] = inp['b5'].reshape(1, 512).astype(np.float32).copy()
    g1 = inp['bng1'].astype(np.float32) * BNI
    d['wfc1'] = np.ascontiguousarray((g1[:, None] * inp['fc1_w']).T)        # (512, 256)
    bf1 = g1 * inp['fc1_b'].astype(np.float32) + inp['bnb1'].astype(np.float32)
    d['bfc1'] = np.ascontiguousarray(bf1.reshape(2, 128).T)                 # (128, 2)
    g2 = inp['bng2'].astype(np.float32) * BNI
    d['wfc2'] = np.ascontiguousarray((g2[:, None] * inp['fc2_w']).T)        # (256, 128)
    bf2 = g2 * inp['fc2_b'].astype(np.float32) + inp['bnb2'].astype(np.float32)
    d['bfc2'] = np.ascontiguousarray(bf2.reshape(128, 1))                   # (128, 1)
    d['wfc3'] = np.ascontiguousarray(inp['fc3_w'].T)                        # (128, 40)
    d['bfc3'] = inp['fc3_b'].reshape(1, 40).astype(np.float32).copy()
    return d


def build_nc(stage='full'):
    # The stock cost model assumes the plain-SWDGE descriptor rate
    # (0.34 ns/desc); dma_gather's per-idx generation measures ~7.5 ns. The
    # Tile scheduler needs the real number to hide gathers behind compute.
    import concourse.hw_specs as hw_specs
    hw_specs.TRN2Spec.SWDGE_NS_PER_DESCRIPTOR = 7.5
    nc = bacc.Bacc("TRN2", target_bir_lowering=False, debug=False, num_devices=8,
                   num_swdge_queues=NQ)
    with TileContext(nc) as tc:
        _trace(nc, tc, stage)
    nc.compile()
    return nc


def _trace(nc, tc, stage='full'):
    with contextlib.ExitStack() as ctx:
        dram = ctx.enter_context(tc.tile_pool(name="dram", bufs=1, space="DRAM"))
        consts = ctx.enter_context(tc.tile_pool(name="consts", bufs=1))
        persist = ctx.enter_context(tc.tile_pool(name="persist", bufs=1))
        sb = ctx.enter_context(tc.tile_pool(name="sb", bufs=2))
        keyp = ctx.enter_context(tc.tile_pool(name="keyp", bufs=2))
        gath = ctx.enter_context(tc.tile_pool(name="gath", bufs=3))
        mp = ctx.enter_context(tc.tile_pool(name="mp", bufs=1))
        scr = ctx.enter_context(tc.tile_pool(name="scr", bufs=2))
        psb = ctx.enter_context(tc.tile_pool(name="psb", bufs=3, space="PSUM"))
        pss = ctx.enter_context(tc.tile_pool(name="pss", bufs=2, space="PSUM"))

        # ---- DRAM I/O ----
        x_d = dram.tile([3, N], F32, kind="ExternalInput", uniquify=False, name="x")
        win = {}
        for li, (C, O) in enumerate(LAYERS, start=1):
            wdtr = C + 1 if C + 2 <= 128 else C
            win[f'wnt{li}'] = dram.tile([C, O], F32, kind="ExternalInput", uniquify=False, name=f"wnt{li}")
            win[f'wdt{li}'] = dram.tile([wdtr, O], F32, kind="ExternalInput", uniquify=False, name=f"wdt{li}")
            win[f'bs{li}'] = dram.tile([1, O], F32, kind="ExternalInput", uniquify=False, name=f"bs{li}")
        w5t_d = dram.tile([512, 512], F32, kind="ExternalInput", uniquify=False, name="w5t")
        w5th_d = dram.tile([256, 512], F16, kind="ExternalInput", uniquify=False, name="w5th")
        b5_d = dram.tile([1, 512], F32, kind="ExternalInput", uniquify=False, name="b5")
        wfc1_d = dram.tile([512, 256], F32, kind="ExternalInput", uniquify=False, name="wfc1")
        bfc1_d = dram.tile([128, 2], F32, kind="ExternalInput", uniquify=False, name="bfc1")
        wfc2_d = dram.tile([256, 128], F32, kind="ExternalInput", uniquify=False, name="wfc2")
        bfc2_d = dram.tile([128, 1], F32, kind="ExternalInput", uniquify=False, name="bfc2")
        wfc3_d = dram.tile([128, 40], F32, kind="ExternalInput", uniquify=False, name="wfc3")
        bfc3_d = dram.tile([1, 40], F32, kind="ExternalInput", uniquify=False, name="bfc3")
        out_d = dram.tile([40, 1], F32, kind="ExternalOutput", uniquify=False, name="out")
        dbg_d = None
        if stage != 'full':
            dbg_d = dram.tile([128, N], F32, kind="ExternalOutput", uniquify=False, name="dbg")

        a_ds = {li: dram.tile([N, O], F16 if li >= 3 else F32, name=f"a_d{li}")
                for li, (C, O) in enumerate(LAYERS, start=1)}
        jw_ds = {li: dram.tile([N * KNN // 16, 128], I16, name=f"jw_d{li}")
                 for li in range(1, 5)}

        # ---- consts ----
        iotaJ = consts.tile([128, N], U32, tag="iotaJ")
        nc.gpsimd.iota(iotaJ[:, :], [[1, N]], base=0, channel_multiplier=0)
        ident = consts.tile([128, 128], F32, tag="ident")
        make_identity(nc, ident[:, :])
        onescol = consts.tile([128, 1], F32, tag="onescol")
        nc.vector.memset(onescol[:, :], 1.0)
        onesrow = consts.tile([1, N], F32, tag="onesrow")
        nc.vector.memset(onesrow[:, :], 1.0)
        maskc = consts.tile([128, 1], U32, tag="maskc")
        nc.vector.memset(maskc[:, :], 0xFFFFFC00)
        onesrow16 = consts.tile([1, N], F16, tag="onesrow16")
        nc.vector.memset(onesrow16[:, :], 1.0)

        # persistent feature tensors; layers with C+2 <= 128 carry two extra
        # rows [ones; -S] so the pd rank-2 terms fuse into the main matmul
        x0T = persist.tile([5, N], F32, tag="x0T")
        x1T = persist.tile([66, N], F32, tag="x1T")
        x2T = persist.tile([66, N], F32, tag="x2T")
        x3T = persist.tile([128, N], F32, tag="x3T")
        x4Ta = persist.tile([128, N], F16, tag="x4Ta")
        x4Tb = persist.tile([128, N], F16, tag="x4Tb")
        nc.sync.dma_start(x0T[3:4, :], onesrow[0:1, :])
        nc.sync.dma_start(x1T[64:65, :], onesrow[0:1, :])
        nc.sync.dma_start(x2T[64:65, :], onesrow[0:1, :])

        # x is pre-transposed on host to (3, N)
        nc.sync.dma_start(x0T[0:3, :], x_d[:, :])
        if stage == 'xload':
            nc.sync.dma_start(dbg_d[0:3, :], x0T[:, :])
            nc.sync.dma_start(out_d[:, :], onescol[0:40, :])
            return

        # weight loads are emitted per-layer via load_w: L1 up-front, the
        # rest inside the previous layer's gather phase so the sync engine's
        # DMA queue never delays a layer-head critical transfer
        wsb = {}

        def load_w(li):
            C, O = LAYERS[li - 1]
            fuse = C + 2 <= 128
            wnt = consts.tile([C, O], F32, tag=f"wnt{li}s")
            wdt = consts.tile([C + 1 if fuse else C, O], F32, tag=f"wdt{li}s")
            bs = consts.tile([1, O], F32, tag=f"bs{li}s")
            nc.sync.dma_start(wnt[:, :], win[f'wnt{li}'][:, :])
            nc.sync.dma_start(wdt[:, :], win[f'wdt{li}'][:, :])
            nc.sync.dma_start(bs[:, :], win[f'bs{li}'][:, :])
            wsb[li] = (wnt, wdt, bs)

        load_w(1)

        def edge_layer(li, xT, C, O, out_parts, post_tiles=None,
                       post_head=None):
            fuse = C + 2 <= 128
            a_d = a_ds[li]
            jw_d = jw_ds[li]
            jwap = jw_d[:, :]
            wnt, wdt, bs = wsb[li]

            xsq = sb.tile([C, N], F32, tag="xsq")
            for hh in range(2):
                hcols = slice(hh * 512, (hh + 1) * 512)
                nc.scalar.activation(xsq[:, hcols], xT[0:C, hcols], AF.Square)
            # x2e = [2x; -S; 1] (fused, fp32) or [2x] (L4, fp16: the L4 pd
            # matmuls run in fp16 for 4x PE rate; keys still pack from the
            # fp32 PSUM accumulation)
            x2e = sb.tile([C + 2 if fuse else C, N],
                          F32 if fuse else F16, tag="x2d")
            for hh in range(2):
                hcols = slice(hh * 512, (hh + 1) * 512)
                nc.scalar.activation(
                    x2e[0:C, hcols], xT[0:C, hcols], AF.Copy, bias=0.0, scale=2.0)
            negS = mp.tile([1, N], F32, tag="negS")
            if fuse:
                nc.sync.dma_start(x2e[C + 1:C + 2, :], onesrow[0:1, :])
            else:
                SO2 = mp.tile([2, N], F16, tag="SO2")
                OS2 = mp.tile([2, N], F16, tag="OS2")
                nc.vector.memset(OS2[0:1, :], -1.0)
                x3h = sb.tile([128, N], F16, tag="x3h")
                for hh in range(2):
                    hcols = slice(hh * 512, (hh + 1) * 512)
                    nc.scalar.activation(
                        x3h[:, hcols], xT[0:C, hcols], AF.Copy)
                negS16 = mp.tile([1, N], F16, tag="negS16")
            for h in range(2):
                cols = slice(h * 512, (h + 1) * 512)
                S_ps = pss.tile([1, 512], F32, tag="a")
                nc.tensor.matmul(
                    S_ps[:, :], lhsT=onescol[0:C, :], rhs=xsq[:, cols],
                    start=True, stop=True, skip_group_check=True)
                nc.scalar.activation(
                    negS[0:1, cols], S_ps[:, :], AF.Copy, bias=0.0, scale=-1.0)
                if not fuse:
                    nc.scalar.activation(
                        SO2[0:1, cols], S_ps[:, :], AF.Copy, bias=0.0, scale=1.0)
            if fuse:
                # compute engines can only address partition bases 0/32/64/96;
                # rows C/C+1 land elsewhere, so move -S in via DMA
                nc.sync.dma_start(xT[C + 1:C + 2, :], negS[0:1, :])
                if C % 32 == 0:
                    nc.scalar.activation(
                        x2e[C:C + 1, :], negS[0:1, :], AF.Copy)
                else:
                    nc.sync.dma_start(x2e[C:C + 1, :], negS[0:1, :])
            else:
                nc.scalar.activation(negS16[0:1, :], negS[0:1, :], AF.Copy)
                nc.sync.dma_start(SO2[1:2, :], onesrow16[0:1, :])
                nc.sync.dma_start(OS2[1:2, :], negS16[0:1, :])

            # a-rows to DRAM first so gathers can start as soon as idx ready
            for t in range(NT):
                a_ps = pss.tile([128, O], F32, tag="a")
                nc.tensor.matmul(
                    a_ps[:, :], lhsT=xT[0:C, t * 128:(t + 1) * 128], rhs=wnt[:, :],
                    start=True, stop=True, skip_group_check=True)
                a_sb = sb.tile([128, O], F16 if li >= 3 else F32, tag="a_sb")
                nc.scalar.activation(a_sb[:, :], a_ps[:, :], AF.Copy)
                nc.sync.dma_start(a_d[t * 128:(t + 1) * 128, :], a_sb[:, :])

            if post_head is not None:
                post_head()

            m = mp.tile([128, NT, O], F32, tag="m")
            pending_g = []


            def emit_reduce(tp, gp, koff):
                if li >= 3:
                    # fp16: contiguous TT max tree (2x DVE mode), in place
                    t10 = sb.tile([128, 10, O], F16, tag="t10", bufs=2)
                    nc.vector.tensor_tensor(
                        out=t10[:, :, :], in0=gp[:, koff:koff + 10, :],
                        in1=gp[:, koff + 10:koff + 20, :], op=ALU.max)
                    nc.vector.tensor_tensor(
                        out=t10[:, 0:5, :], in0=t10[:, 0:5, :], in1=t10[:, 5:10, :],
                        op=ALU.max)
                    nc.vector.tensor_tensor(
                        out=t10[:, 0:2, :], in0=t10[:, 0:2, :], in1=t10[:, 2:4, :],
                        op=ALU.max)
                    nc.vector.tensor_tensor(
                        out=t10[:, 0, :], in0=t10[:, 0, :], in1=t10[:, 1, :],
                        op=ALU.max)
                    nc.vector.tensor_tensor(
                        out=m[:, tp, :], in0=t10[:, 0, :], in1=t10[:, 4, :],
                        op=ALU.max)
                    return
                gap = gp[:, :, :]
                red0 = bass.AP(gap.tensor, gap.offset + koff * O,
                               [gap.ap[0], [1, O], [O, KNN // 2]])
                red1 = bass.AP(gap.tensor, gap.offset + (koff + KNN // 2) * O,
                               [gap.ap[0], [1, O], [O, KNN // 2]])
                mtmp = sb.tile([128, O], F32, tag="mtmp", bufs=2)
                nc.vector.tensor_reduce(
                    out=m[:, tp, :], in_=red0, axis=AX.X, op=ALU.max)
                nc.vector.tensor_reduce(
                    out=mtmp[:, :], in_=red1, axis=AX.X, op=ALU.max)
                nc.vector.tensor_tensor(
                    out=m[:, tp, :], in0=m[:, tp, :], in1=mtmp[:, :],
                    op=ALU.max)

            def emit_pair(tp0, tp1, gp):
                emit_reduce(tp0, gp, 0)
                emit_reduce(tp1, gp, KNN)
            ntl = NT
            if stage.startswith('topk1_'):
                ntl = int(stage.split('_')[1])
            for t in range(ntl):
                if stage in ('keys', 'pack', 'max1', 'mr1', 'ext', 'topkt0') and t > 0:
                    break
                tcols = slice(t * 128, (t + 1) * 128)
                kp = psb.tile([128, N], F32, tag="big")
                for h in range(2):
                    cols = slice(h * 512, (h + 1) * 512)
                    if fuse:
                        nc.tensor.matmul(
                            kp[:, cols], lhsT=xT[:, tcols], rhs=x2e[:, cols],
                            start=True, stop=True, skip_group_check=True)
                    else:
                        nc.tensor.matmul(
                            kp[:, cols], lhsT=x3h[:, tcols], rhs=x2e[:, cols],
                            start=True, stop=False, skip_group_check=True)
                        nc.tensor.matmul(
                            kp[:, cols], lhsT=SO2[:, tcols], rhs=OS2[:, cols],
                            start=False, stop=True, skip_group_check=True)
                if stage == 'keys' and t == 0:
                    kcp = sb.tile([128, N], F32, tag="kcp")
                    nc.scalar.activation(kcp[:, :], kp[:, :], AF.Copy)
                    nc.sync.dma_start(dbg_d[:, :], kcp[:, :])
                    nc.sync.dma_start(out_d[:, :], onescol[0:40, :])
                    return 'stop'
                kb = keyp.tile([128, N], U32, tag="keysP")
                nc.vector.scalar_tensor_tensor(
                    out=kb[:, :], in0=kp[:, :].bitcast(U32), scalar=maskc[:, 0:1],
                    in1=iotaJ[:, :], op0=ALU.bitwise_and, op1=ALU.bitwise_or)
                if stage == 'pack' and t == 0:
                    kcp = sb.tile([128, N], F32, tag="kcp")
                    nc.vector.tensor_copy(kcp[:, :].bitcast(U32), kb[:, :])
                    nc.sync.dma_start(dbg_d[:, :], kcp[:, :])
                    nc.sync.dma_start(out_d[:, :], onescol[0:40, :])
                    return 'stop'
                kbf = kb[:, :].bitcast(F32)
                v64 = scr.tile([128, 64], F32, tag="v64")
                for ch in range(8):
                    nc.vector.max(
                        v64[:, ch * 8:(ch + 1) * 8],
                        kb[:, ch * 128:(ch + 1) * 128].bitcast(F32))
                v24 = scr.tile([128, 24], F32, tag="v24")
                nc.vector.max(v24[:, 0:8], v64[:, :])
                if stage == 'max1' and t == 0:
                    nc.sync.dma_start(dbg_d[:, 0:8], v24[:, 0:8])
                    nc.sync.dma_start(out_d[:, :], onescol[0:40, :])
                    return 'stop'
                v64f = v64[:, :]
                nc.vector.match_replace(v64f, v24[:, 0:8], v64f, NEG_BIG)
                if stage == 'mr1' and t == 0:
                    nc.sync.dma_start(dbg_d[:, :], kb[:, :].bitcast(F32))
                    nc.sync.dma_start(out_d[:, :], onescol[0:40, :])
                    return 'stop'
                nc.vector.max(v24[:, 8:16], v64f)
                nc.vector.match_replace(v64f, v24[:, 8:16], v64f, NEG_BIG)
                nc.vector.max(v24[:, 16:24], v64f)
                if stage == 'ext' and t == 0:
                    nc.sync.dma_start(dbg_d[:, 0:24], v24[:, :])
                    nc.sync.dma_start(out_d[:, :], onescol[0:40, :])
                    return 'stop'
                j20 = scr.tile([128, KNN], U32, tag="j20")
                nc.vector.tensor_scalar(
                    j20[:, :], v24[:, 0:KNN].bitcast(U32), 0x3FF, None,
                    op0=ALU.bitwise_and)
                # j as fp32 values; two tiles share one jf2 so the whole idx
                # chain (transpose/replicate/DRAM wrap/gather) runs once per
                # tile PAIR, halving per-tile sequencer round-trips
                if t % 2 == 0:
                    jf2 = scr.tile([128, 2, KNN], F32, tag="jf", bufs=2)
                nc.vector.tensor_copy(jf2[:, t % 2, :], j20[:, :])
                if stage == 'topkt0' and t == 0:
                    nc.sync.dma_start(dbg_d[:, 0:KNN], jf2[:, 0, :])
                    nc.sync.dma_start(out_d[:, :], onescol[0:40, :])
                    return 'stop'
                if t % 2 == 0:
                    continue

                # pair idx wrap + 4 full-size gathers (one per queue)
                jT_ps = pss.tile([2 * KNN, 128], F32, tag="a")
                nc.tensor.matmul(
                    jT_ps[:, :], lhsT=jf2[:, :, :].rearrange("p a k -> p (a k)"),
                    rhs=ident[:, 0:128], is_transpose=True, start=True, stop=True,
                    skip_group_check=True)
                # build the 8x-replicated row layout in one activation via a
                # stride-0 (broadcast) read dim, then a single DMA write
                jTi8 = sb.tile([2 * KNN, 1024], I16, tag="jTi8", bufs=2)
                pap = jT_ps[:, :]
                rep_src = bass.AP(pap.tensor, pap.offset,
                                  [pap.ap[0], [16, 8], [0, 8], [1, 16]])
                nc.scalar.activation(jTi8[:, :], rep_src, AF.Copy)
                dst = bass.AP(jwap.tensor, jwap.offset + (t - 1) * 160 * 128,
                              [[1024, 2 * KNN], [1, 1024]])
                nc.sync.dma_start(dst, jTi8[:, :])
                src_ap = bass.AP(jwap.tensor, jwap.offset + (t - 1) * 160 * 128,
                                 [[128, 320], [1, 128]])
                idq = keyp.tile([128, 320], I16, tag="idxq", bufs=2)
                nc.scalar.dma_start_transpose(idq[:, :], src_ap)

                gdt = F16 if li >= 3 else F32
                g = gath.tile([128, 2 * KNN, O], gdt, tag="g", bufs=3)
                # 4 gathers per pair: (tile, k-half) x (2, 2), one per queue;
                # each idx range is contiguous in idq
                for gh in range(4):
                    tt, kh = gh // 2, gh % 2
                    nc.gpsimd.dma_gather(
                        out_ap=g[:, tt * KNN + kh * 10:tt * KNN + kh * 10 + 10, :],
                        in_ap=a_d[:, :],
                        idxs_ap=idq[:, tt * 160 + kh * 80:tt * 160 + kh * 80 + 80],
                        num_idxs=1280, num_idxs_reg=1280, elem_size=O,
                        single_packet=False,
                        queue_num=(2 * t + gh) % NQ)
                pending_g.append((t - 1, t, g))
                # emit layer-independent PE work (zpart_fill) mid-loop so
                # the scheduler anchors it INSIDE the gather phase
                if post_tiles is not None and t == 3:
                    post_tiles()
                    post_tiles = None
                # lag the reduce a couple pairs so the DVE stream never
                # head-of-line blocks on an in-flight gather
                if len(pending_g) > 2:
                    emit_pair(*pending_g.pop(0))

            for tp0, tp1, gp in pending_g:
                emit_pair(tp0, tp1, gp)

            if stage.startswith('topk1'):
                nc.sync.dma_start(dbg_d[0:128, 0:KNN], jf2[:, 1, :])
                nc.sync.dma_start(out_d[:, :], onescol[0:40, :])
                return 'stop'

            if post_tiles is not None:
                post_tiles()

            # transpose m + c matmul + lrelu -> out_parts, pipelined by
            # column halves so the next layer's head can start on half 0
            # while half 1 still accumulates
            for ot, (otile, orow) in enumerate(out_parts):
                px = psb.tile([orow, N], F32, tag="big")
                for h in range(2):
                    cols = slice(h * 512, (h + 1) * 512)
                    for t in range(4 * h, 4 * h + 4):
                        nc.tensor.matmul(
                            px[:, t * 128:(t + 1) * 128],
                            lhsT=m[:, t, ot * 128:ot * 128 + orow],
                            rhs=ident[:, 0:128],
                            is_transpose=True, start=(t % 4 == 0), stop=False,
                            skip_group_check=True)
                    if fuse:
                        # wdt carries the bias as row C against xT's ones row
                        nc.tensor.matmul(
                            px[:, cols],
                            lhsT=wdt[:, ot * 128:ot * 128 + orow],
                            rhs=xT[0:C + 1, cols],
                            start=False, stop=True, skip_group_check=True)
                    else:
                        nc.tensor.matmul(
                            px[:, cols],
                            lhsT=wdt[:, ot * 128:ot * 128 + orow],
                            rhs=xT[:, cols],
                            start=False, stop=False, skip_group_check=True)
                        nc.tensor.matmul(
                            px[:, cols],
                            lhsT=bs[0:1, ot * 128:ot * 128 + orow],
                            rhs=onesrow[0:1, cols],
                            start=False, stop=True, skip_group_check=True)
                    nc.scalar.activation(
                        otile[0:orow, cols], px[:, cols], AF.Prelu,
                        alpha=NEG_SLOPE)

        w5sb = {}
        b5sb_box = {}

        def load_w5():
            for ci, (rows, k0) in enumerate([(64, 0), (64, 64), (128, 128)]):
                w5c = consts.tile([rows, 512], F32, tag=f"w5c{ci}")
                nc.sync.dma_start(w5c[:, :], w5t_d[k0:k0 + rows, :])
                w5sb[ci] = w5c
            for ci, k0 in ((3, 0), (4, 128)):
                w5c = consts.tile([128, 512], F16, tag=f"w5c{ci}")
                nc.sync.dma_start(w5c[:, :], w5th_d[k0:k0 + 128, :])
                w5sb[ci] = w5c
            b5sb = consts.tile([1, 512], F32, tag="b5sb")
            nc.sync.dma_start(b5sb[:, :], b5_d[:, :])
            b5sb_box[0] = b5sb

        fcsb = {}

        def load_fc():
            wfc1sb = consts.tile([128, 4, 256], F32, tag="wfc1sb")
            for c in range(4):
                nc.sync.dma_start(wfc1sb[:, c, :],
                                  wfc1_d[c * 128:(c + 1) * 128, :])
            bfc1sb = consts.tile([128, 2], F32, tag="bfc1sb")
            nc.sync.dma_start(bfc1sb[:, :], bfc1_d[:, :])
            wfc2sb = consts.tile([128, 2, 128], F32, tag="wfc2sb")
            for c in range(2):
                nc.sync.dma_start(wfc2sb[:, c, :],
                                  wfc2_d[c * 128:(c + 1) * 128, :])
            bfc2sb = consts.tile([128, 1], F32, tag="bfc2sb")
            nc.sync.dma_start(bfc2sb[:, :], bfc2_d[:, :])
            wfc3sb = consts.tile([128, 40], F32, tag="wfc3sb")
            nc.sync.dma_start(wfc3sb[:, :], wfc3_d[:, :])
            bfc3sb = consts.tile([1, 40], F32, tag="bfc3sb")
            nc.sync.dma_start(bfc3sb[:, :], bfc3_d[:, :])
            fcsb.update(wfc1sb=wfc1sb, bfc1sb=bfc1sb, wfc2sb=wfc2sb,
                        bfc2sb=bfc2sb, wfc3sb=wfc3sb, bfc3sb=bfc3sb)

        zpart = persist.tile([128, NT, 512], F32, tag="zpart")

        def zpart_fill():
            for t in range(NT):
                tcols = slice(t * 128, (t + 1) * 128)
                zp_ps = pss.tile([128, 512], F32, tag="a")
                for ci, (xt, rows) in enumerate(
                        [(x1T, 64), (x2T, 64), (x3T, 128)]):
                    nc.tensor.matmul(
                        zp_ps[:, :], lhsT=xt[0:rows, tcols], rhs=w5sb[ci][:, :],
                        start=(ci == 0), stop=False, skip_group_check=True)
                # conv5 bias folded in here (hidden under L4) instead of in
                # the head's critical path
                nc.tensor.matmul(
                    zp_ps[:, :], lhsT=onesrow[0:1, tcols],
                    rhs=b5sb_box[0][:, :], start=False, stop=True,
                    skip_group_check=True)
                nc.scalar.activation(zpart[:, t, :], zp_ps[:, :], AF.Copy)

        load_w(2)
        load_w(3)
        load_w(4)
        load_w5()
        r = edge_layer(1, x0T, 3, 64, [(x1T, 64)])
        if r == 'stop':
            return
        if stage == 'gath1':
            nc.sync.dma_start(dbg_d[0:64, :], x1T[0:64, :])
            nc.sync.dma_start(out_d[:, :], onescol[0:40, :])
            return
        edge_layer(2, x1T, 64, 64, [(x2T, 64)])
        edge_layer(3, x2T, 64, 128, [(x3T, 128)])
        edge_layer(4, x3T, 128, 256, [(x4Ta, 128), (x4Tb, 128)],
                   post_tiles=zpart_fill)

        # ---- head: conv5 (x4 chunks; x1-x3 partials precomputed) + max pool ----
        zmax = persist.tile([128, 512], F32, tag="zmax")
        for t in range(NT):
            tcols = slice(t * 128, (t + 1) * 128)
            z_ps = pss.tile([128, 512], F32, tag="a")
            for ci, (xt, rows, k0) in enumerate(
                    [(x4Ta, 128, 256), (x4Tb, 128, 384)]):
                nc.tensor.matmul(
                    z_ps[:, :], lhsT=xt[:, tcols], rhs=w5sb[3 + ci][:, :],
                    start=(ci == 0), stop=(ci == 1), skip_group_check=True)
            zsb = sb.tile([128, 512], F32, tag="zsb")
            nc.vector.tensor_tensor(
                out=zsb[:, :], in0=zpart[:, t, :], in1=z_ps[:, :], op=ALU.add)
            if t == 0:
                nc.scalar.activation(zmax[:, :], zsb[:, :], AF.Copy)
            else:
                nc.vector.tensor_tensor(
                    out=zmax[:, :], in0=zmax[:, :], in1=zsb[:, :], op=ALU.max)
        # partition tree-max 128 -> 1... then we need yT [128, 4] instead:
        # transpose zmax chunks and reduce along free dim.
        yT = persist.tile([128, 4], F32, tag="yT")
        for cchunk in range(4):
            zt_ps = pss.tile([128, 128], F32, tag="a")
            nc.tensor.matmul(
                zt_ps[:, :], lhsT=zmax[:, cchunk * 128:(cchunk + 1) * 128],
                rhs=ident[:, 0:128], is_transpose=True, start=True, stop=True,
                skip_group_check=True)
            nc.vector.tensor_reduce(
                out=yT[:, cchunk:cchunk + 1], in_=zt_ps[:, :],
                axis=AX.X, op=ALU.max)
        # leaky relu on yT
        yTr = persist.tile([128, 4], F32, tag="yTr")
        nc.scalar.activation(yTr[:, :], yT[:, :], AF.Prelu, alpha=NEG_SLOPE)

        # ---- FC head ----
        load_fc()
        wfc1sb = fcsb['wfc1sb']
        bfc1sb = fcsb['bfc1sb']
        wfc2sb = fcsb['wfc2sb']
        bfc2sb = fcsb['bfc2sb']
        wfc3sb = fcsb['wfc3sb']
        bfc3sb = fcsb['bfc3sb']

        h1sb = persist.tile([128, 2], F32, tag="h1sb")
        for mt in range(2):
            h1_ps = pss.tile([128, 1], F32, tag="a")
            for c in range(4):
                nc.tensor.matmul(
                    h1_ps[:, :], lhsT=wfc1sb[:, c, mt * 128:(mt + 1) * 128],
                    rhs=yTr[:, c:c + 1],
                    start=(c == 0), stop=(c == 3), skip_group_check=True)
            nc.scalar.activation(
                h1sb[:, mt:mt + 1], h1_ps[:, :], AF.Prelu,
                bias=bfc1sb[:, mt:mt + 1], scale=1.0, alpha=NEG_SLOPE)
        h2sb = persist.tile([128, 1], F32, tag="h2sb")
        h2_ps = pss.tile([128, 1], F32, tag="a")
        for c in range(2):
            nc.tensor.matmul(
                h2_ps[:, :], lhsT=wfc2sb[:, c, :], rhs=h1sb[:, c:c + 1],
                start=(c == 0), stop=(c == 1), skip_group_check=True)
        nc.scalar.activation(
            h2sb[:, :], h2_ps[:, :], AF.Prelu,
            bias=bfc2sb[:, :], scale=1.0, alpha=NEG_SLOPE)

        out_ps = pss.tile([40, 1], F32, tag="a")
        nc.tensor.matmul(
            out_ps[:, :], lhsT=wfc3sb[:, :], rhs=h2sb[:, :],
            start=True, stop=False, skip_group_check=True)
        nc.tensor.matmul(
            out_ps[:, :], lhsT=bfc3sb[:, :], rhs=onescol[0:1, :],
            start=False, stop=True, skip_group_check=True)
        out_sb = persist.tile([40, 1], F32, tag="out_sb")
        nc.scalar.activation(out_sb[:, :], out_ps[:, :], AF.Copy)
        nc.sync.dma_start(out_d[:, :], out_sb[:, :])


# ---------------------------------------------------------------------------
# harness entry point
# ---------------------------------------------------------------------------
_NC_CACHE = {}


def _get_nc():
    if 'nc' not in _NC_CACHE:
        _NC_CACHE['nc'] = build_nc()
    return _NC_CACHE['nc']


def kernel(**inputs):
    """Full-batch EdgeCNN forward. x: (8, 1024, 3) -> (8, 40) float32.

    Pure data parallel: batch element b runs on NeuronCore b.
    """
    from concourse.bass_utils import run_bass_kernel_spmd

    inp = {k: np.asarray(v) for k, v in inputs.items()}
    prep = host_prep(inp)
    nc = _get_nc()
    in_maps = []
    for b in range(8):
        m = {'x': np.ascontiguousarray(inp['x'][b].T).astype(np.float32)}
        m.update(prep)
        in_maps.append(m)
    res = run_bass_kernel_spmd(nc, in_maps, core_ids=list(range(8)))
    out = np.stack([res.results[b]['out'].reshape(40) for b in range(8)])
    return out.astype(np.float32)

